# revision 13
# baseline (speedup 1.0000x reference)
"""Bass/Trainium2 kernel for the decomposed LocallyConnected2d layer.

out[b,o,i,j] = sum_{c,k} x[b, c, i+di, j+dj] * w[o, c, i, j, k] + bias[o,i,j]
with k = di*3 + dj (3x3 kernel, stride 1).

Strategy: shard over output rows i across 8 cores (4 rows each). Per output
location (i,j) the contraction (di,c,dj)=288 is split into 3 chunks of 96 =
(di,c), chunked over dj; each chunk is one matmul lhsT=[96,64] rhs=[96,128]
accumulating into PSUM [64 o, 128 b]. Even/odd j use PE column groups 0/1
(tile_position) so two locations' matmuls overlap in the array.

The kernel is HBM-byte-bound, so all weight chunks ship as fp8-e4m3
(scaled x256 to stay out of the subnormal range; measured rel-err 1.66e-2
on the real seed vs the 2e-2 gate). The bias stays exact fp16 and enters
as a PSUM-initializing matmul per (group, column-half): lhsT = [4, 64]
bias slice (rows = the group's 4 j-pair positions), rhs = a constant [4,
4*128] one-hot selector, so out[o, pig, b] = bias[o, j(pig)] fills the
half-group in one instruction. Chunk matmuls then accumulate on top
(start=False) and the dj=2 chunk closes each slice. PSUM->SBUF copies
apply the 1/256 compensation (vector and scalar engines alternate) and the
output leaves as fp16.

DMA design: ~7.8 MB/core against a ~260 GB/s effective DMA ceiling, and an
activity governor that halves the core's speed once it has been busy for
~30us - so the whole kernel must land before ~36us. Inputs stream on the
two HWDGE queues (which drain FIFO) in per-row granularity, balanced
~2.9MB/queue so each row's x slab and weights land at the same time:
scalar = x0,x1,x2,w3; sync = tiny selector+bias, w0,w1,w2,x3. Output rows
leave on the third (gpsimd SWDGE) queue so the writes overlap the input
stream instead of queueing behind it. The HWDGE only spreads a DMA across
the 16 DMA engines when the outermost AP dim is a multiple of 16, so every
bulk DMA is 96 or 128 partitions. 14 DMAs total, tiny ones first so
DMA-completion-sem reuse never chains behind a live transfer.
"""

import sys

for _p in ("/opt/trn_rl_repo", "/root/.axon_site/_ro/trn_rl_repo"):
    if _p not in sys.path:
        sys.path.append(_p)

import numpy as np

B = 128
C_IN = 32
C_OUT = 64
OH = OW = 32
KH = KW = 3
H = W = 34
N_CORES = 8
RPC = OH // N_CORES          # output rows per core = 4
HALO = RPC + KH - 1          # x rows per core = 6
NPAIR = OW // 2              # j-pairs per row = 16
NGRP = 4                     # j-pairs per psum group
GRPS = NPAIR // NGRP         # psum groups per row = 4
WSCALE = 256.0               # weight scale to keep fp8 out of subnormals

_prog_cache = {}


def _build_program():
    import concourse.tile as tile
    from concourse import bacc, mybir

    f16 = mybir.dt.float16
    f8 = mybir.dt.float8e4
    f32 = mybir.dt.float32

    nc = bacc.Bacc("TRN2", target_bir_lowering=False, debug=False,
                   num_devices=N_CORES)

    # Per-core DRAM I/O (host pre-sharded / pre-transposed):
    #   xs  [96, i=4, w=34, b=128] f16   partition p=di*32+c = x row r0+i+di
    #   w   [96, i=4, j=32, dj=3, o=64] f8   all chunks, scaled x256
    #   bias[4, i=4, g=4, par=2, o=64] f16   row k = j-pair slot, scaled x256
    #   sel [4, 4*128] f16               one-hot: sel[k, (pig, b)] = (pig==k)
    #   out [p2=128 (par*64+o), i=4, jh=16, b=128] f16 ; j = 2*jh + par
    xs_in = nc.dram_tensor("xs", [96, RPC, W, B], f16,
                           kind="ExternalInput").ap()
    w_in = nc.dram_tensor("w", [96, RPC, OW, KW, C_OUT], f8,
                          kind="ExternalInput").ap()
    bias_in = nc.dram_tensor("bias", [4, RPC, GRPS, 2, C_OUT], f16,
                             kind="ExternalInput").ap()
    sel_in = nc.dram_tensor("sel", [4, NGRP * B], f16,
                            kind="ExternalInput").ap()
    out = nc.dram_tensor("out", [128, RPC, NPAIR, B], f16,
                         kind="ExternalOutput").ap()

    with tile.TileContext(nc) as tc:
        with (
            tc.tile_pool(name="xpool", bufs=1) as xpool,
            tc.tile_pool(name="wpool", bufs=1) as wpool,
            tc.tile_pool(name="opool", bufs=3) as opool,
            tc.tile_pool(name="pspool", bufs=6, space="PSUM") as pspool,
        ):
            xr = [xpool.tile([96, W, B], f16, tag=f"x{i}", name=f"x{i}")
                  for i in range(RPC)]
            wr = [wpool.tile([96, OW, KW, C_OUT], f8, tag=f"w{i}",
                             name=f"w{i}")
                  for i in range(RPC)]
            selb = wpool.tile([4, NGRP * B], f16, tag="selb")
            biast = wpool.tile([4, RPC, GRPS, 2, C_OUT], f16, tag="biast")

            # tiny DMAs first (they finish immediately), then bulk,
            # interleaved so each queue carries ~2.9MB and row i's x and w
            # land together (x slab 0.84MB, w row 0.59MB).
            nc.sync.dma_start(selb[:], sel_in[:])
            nc.sync.dma_start(biast[:], bias_in[:])
            nc.scalar.dma_start(xr[0][:], xs_in[:, 0])
            nc.sync.dma_start(wr[0][:], w_in[:, 0])
            nc.scalar.dma_start(xr[1][:], xs_in[:, 1])
            nc.sync.dma_start(wr[1][:], w_in[:, 1])
            nc.scalar.dma_start(xr[2][:], xs_in[:, 2])
            nc.sync.dma_start(wr[2][:], w_in[:, 2])
            nc.scalar.dma_start(wr[3][:], w_in[:, 3])
            nc.sync.dma_start(xr[3][:], xs_in[:, 3])

            inv_s = 1.0 / WSCALE
            for i in range(RPC):
                orow = opool.tile([128, NPAIR, B], f16, tag=f"o{i}",
                                  name=f"o{i}")
                wt = wr[i]
                xt = xr[i]
                for g in range(GRPS):
                    ps = pspool.tile([128, NGRP, B], f32)
                    for par in range(2):
                        # PSUM init: the half-group becomes bias[o, j(pig)].
                        nc.tensor.matmul(ps[64 * par:64 * par + 64, :, :],
                                         biast[0:4, i, g, par, :],
                                         selb[0:4, :],
                                         start=True, stop=False,
                                         tile_position=(0, 64 * par))
                    for pig in range(NGRP):
                        for par in range(2):
                            j = 2 * (NGRP * g + pig) + par
                            pslice = ps[64 * par:64 * par + 64, pig, :]
                            tp = (0, 64 * par)
                            nc.tensor.matmul(pslice, wt[:, j, 0, :],
                                             xt[:, j, :],
                                             start=False, stop=False,
                                             tile_position=tp)
                            nc.tensor.matmul(pslice, wt[:, j, 1, :],
                                             xt[:, j + 1, :],
                                             start=False, stop=False,
                                             tile_position=tp)
                            nc.tensor.matmul(pslice, wt[:, j, 2, :],
                                             xt[:, j + 2, :],
                                             start=False, stop=True,
                                             tile_position=tp)
                    dst = orow[:, NGRP * g:NGRP * (g + 1), :]
                    if g % 2 == 0:
                        nc.vector.tensor_scalar_mul(dst, ps[:], inv_s)
                    else:
                        nc.scalar.mul(dst, ps[:], inv_s)
                nc.gpsimd.dma_start(out[:, i], orow[:])

    nc.compile()
    return nc


def _host_prep(x, weight, bias):
    """Full fp32 inputs -> list of per-core input dicts."""
    import ml_dtypes
    f8 = ml_dtypes.float8_e4m3

    # x: (B, C, H, W) -> (C, H, W, B) fp16
    x_t = np.ascontiguousarray(x.transpose(1, 2, 3, 0)).astype(np.float16)
    # w: (O, C, I, J, K) with K=(di*3+dj) -> [(di*32+c)=96, I, J, dj, O]
    w_r = (weight * WSCALE).reshape(C_OUT, C_IN, OH, OW, KH, KW)
    w_t = w_r.transpose(4, 1, 2, 3, 5, 0)          # (di, c, I, J, dj, O)
    w_full = w_t.reshape(96, OH, OW, KW, C_OUT).astype(f8)
    # bias: (O, I, J) -> [k=j-pair slot, I, g, par, O]
    b_t = (bias * WSCALE).transpose(1, 2, 0)       # (I, J, O)
    b_t = b_t.reshape(OH, GRPS, NGRP, 2, C_OUT)    # (I, g, k, par, O)
    b_t = np.ascontiguousarray(b_t.transpose(2, 0, 1, 3, 4))  # (k,I,g,par,O)
    b_t = b_t.astype(np.float16)
    # selector: sel[k, (pig, b)] = 1.0 iff pig == k
    sel = np.zeros((4, NGRP, B), np.float16)
    for k in range(4):
        sel[k, k, :] = 1.0
    sel = sel.reshape(4, NGRP * B)

    in_maps = []
    for m in range(N_CORES):
        r0 = m * RPC
        xs = np.empty((96, RPC, W, B), np.float16)
        xsv = xs.reshape(KH, C_IN, RPC, W, B)
        for di in range(KH):
            xsv[di] = x_t[:, r0 + di:r0 + di + RPC]
        in_maps.append({
            "xs": xs,
            "w": np.ascontiguousarray(w_full[:, r0:r0 + RPC]),
            "bias": np.ascontiguousarray(b_t[:, r0:r0 + RPC]),
            "sel": sel,
        })
    return in_maps


def _gather(results):
    out_full = np.empty((B, C_OUT, OH, OW), np.float32)
    for m in range(N_CORES):
        r = results[m]["out"].astype(np.float32)          # (128, 4, 16, 128)
        r = r.reshape(2, C_OUT, RPC, NPAIR, B)            # par,o,i,jh,b
        r = r.transpose(4, 1, 2, 3, 0)                    # b,o,i,jh,par
        out_full[:, :, m * RPC:(m + 1) * RPC, :] = r.reshape(B, C_OUT, RPC, OW)
    return out_full


def kernel(x, weight, bias, _trace=False):
    from concourse.bass_utils import run_bass_kernel_spmd

    if "nc" not in _prog_cache:
        _prog_cache["nc"] = _build_program()
    nc = _prog_cache["nc"]

    in_maps = _host_prep(np.asarray(x), np.asarray(weight), np.asarray(bias))
    res = run_bass_kernel_spmd(nc, in_maps, core_ids=list(range(N_CORES)),
                               trace=_trace)
    out = _gather(res.results)
    if _trace:
        _prog_cache["last_result"] = res
    return out
